# revision 10
# baseline (speedup 1.0000x reference)
"""MultiHeadAttention (B=2, S=2048, D=1024, H=16, dk=dv=64) on 8 trn2 cores.

Head-parallel: core c owns heads (2c, 2c+1). The reference's odd
reshape(B,-1,H*DV) means output row m draws only from head m//256, so the
final fc is fully local per core; host just concatenates.

Math transformations (exact, softmax-invariant):
  - bk dropped: adds a per-query constant to scores -> softmax unchanged.
  - bv folded into the output bias: softmax rows sum to 1, so
    att = w@v0 + bv; downstream y += tile16(bv) @ Wo, precomputed on host.
  - softmax without max-subtraction: scores ~ N(0,1), exp is safe in fp32.

v3 device dataflow per core (all fp16 streams, f32 psum):
  A  (per pair): qT/kT [128=2x64 feats, 2048] = W.T @ actT (+bq for q).
  A' (per pair): v -> natural [t, v] via [128,128] PE transposes; stored
      head-interleaved in vnat[:, tt, 0:128] (v_h at col 2i+h) with ones
      at cols 128,129 so lhsT = vnat[:, tt, h::2] is [t, 65] = {v_h|1}.
  B  (per pair, per 512-col s-quarter): per t-tile: scoresT for both heads
      into one [128,1024] psum (two K=64 matmuls at tile_position rows 0/64
      -> HW-concurrent); one Exp ACT op; at_h[65,512] += vnat.T@ex
      (row 64 = raw softmax denominator). Then per head: at->u copy,
      in-place fp16 reciprocal of the denominator row, gpsimd
      partition_broadcast -> bcs[128,512], odd-column partition-shift via
      eye-matmul, and two multiplies (gpsimd even / DVE odd) that
      normalize AND repack into attP[128, rr, j0] (full-contract layout
      for the fc: rows 0:64 = even s2 = j even, 64:128 = odd).
  C  (per pair): y[rr, o] = sum_j0 attP[:, :, j0].T @ woP[:, j0, o]
      (contract 128 = two j's at once) + bias-mm; DMA from psum to DRAM.
  Pipelining: activations for pair i+1 prefetch during B(i) (works across
  the rep boundary too); emit_C(i-1) is interleaved into B(i) at sq 2/3.
"""

import numpy as np

import concourse.bacc as bacc
import concourse.mybir as mybir
import concourse.tile as tile

B, S, D, H, DK = 2, 2048, 1024, 16, 64
NCORES = 8
KT = D // 128  # 8 contraction tiles
TT = S // 128  # 16 t-tiles
SQ = S // 512  # 4 s-quarters
F32R = mybir.dt.float32r
F32 = mybir.dt.float32
F16 = mybir.dt.float16
AD = F16
NP_AD = np.float16
AF = mybir.ActivationFunctionType


def build_nc(reps=1):
    nc = bacc.Bacc(trn_type="TRN2")

    qT = nc.declare_dram_parameter("qT", [B, 128, KT, S], AD, isOutput=False)
    kTd = nc.declare_dram_parameter("kT", [B, 128, KT, S], AD, isOutput=False)
    vTd = nc.declare_dram_parameter("vT", [B, 128, KT, S], AD, isOutput=False)
    wq = nc.declare_dram_parameter("wq", [128, KT, 128], AD, isOutput=False)
    wk = nc.declare_dram_parameter("wk", [128, KT, 128], AD, isOutput=False)
    wv = nc.declare_dram_parameter("wv", [128, KT, 128], AD, isOutput=False)
    bqd = nc.declare_dram_parameter("bq", [128, 1], F32, isOutput=False)
    eyed = nc.declare_dram_parameter("eye", [128, 128], AD, isOutput=False)
    onesd = nc.declare_dram_parameter("ones", [128, 128], AD, isOutput=False)
    wo = nc.declare_dram_parameter("wo", [128, 8, 1024], AD, isOutput=False)
    bo2 = nc.declare_dram_parameter("bo2", [2, 1024], AD, isOutput=False)
    y = nc.declare_dram_parameter("y", [2, B, 128, 1024], F16, isOutput=True)

    with tile.TileContext(nc) as tc:
        with (
            tc.tile_pool(name="const", bufs=1) as constp,
            tc.tile_pool(name="wts", bufs=1) as wtsp,
            tc.tile_pool(name="acts", bufs=4) as actsp,
            tc.tile_pool(name="proj", bufs=2) as projp,
            tc.tile_pool(name="vaugp", bufs=2) as vaugp,
            tc.tile_pool(name="exp", bufs=6) as expp,
            tc.tile_pool(name="attp", bufs=4) as attp,
            tc.tile_pool(name="attpk", bufs=4) as attpk,
            tc.tile_pool(name="small", bufs=3) as smallp,
            tc.tile_pool(name="wop", bufs=1) as wop,
            tc.tile_pool(name="ps", bufs=1, space="PSUM") as ps,
        ):
            # constants
            eye_sb = constp.tile([128, 128], AD, tag="eye")
            nc.sync.dma_start(out=eye_sb, in_=eyed[:, :])
            ones_sb = constp.tile([128, 128], AD, tag="ones_sb")
            nc.sync.dma_start(out=ones_sb, in_=onesd[:, :])
            bq_sb = constp.tile([128, 1], F32, tag="bq")
            nc.sync.dma_start(out=bq_sb, in_=bqd[:, :])
            bo_sb = constp.tile([1, 2, 1024], AD, tag="bo")
            nc.sync.dma_start(out=bo_sb, in_=bo2[None, :, :])

            # packed per-head weights, resident
            w_sb = {}
            for name, dram in (("q", wq), ("k", wk), ("v", wv)):
                w_sb[name] = wtsp.tile([128, KT, 128], AD, tag="w" + name, name="w" + name)
                nc.sync.dma_start(out=w_sb[name], in_=dram[:, :, :])

            # fc weights resident across reps (like wq/wk/wv)
            woP = wop.tile([128, 8, 1024], AD, tag="woP", name="woP")
            for j in range(2):
                nc.sync.dma_start(
                    out=woP[:, 4 * j : 4 * j + 4, :], in_=wo[:, 4 * j : 4 * j + 4, :]
                )

            npairs = reps * B
            attTs = {}  # (hl, i) -> [65, S]: rows 0-63 unnormalized attT,
            # row 64 = raw softmax denominator (reciprocal'd in place)
            attPs = {}  # (hl, i) -> [128, 128, 8] normalized+repacked

            dram_of = {"q": qT, "k": kTd, "v": vTd}
            dma_eng = {"q": nc.sync, "k": nc.sync, "v": nc.gpsimd}

            def emit_C(ci, hls=(0, 1)):
                # ---------------- phase C: output fc ----------------
                cb = ci % B
                for hl in hls:
                    ysb = smallp.tile([128, 1024], F16, tag="ysb", name=f"ysb{hl}")
                    for ob in range(2):
                        osl = slice(ob * 512, (ob + 1) * 512)
                        yp = ps.tile(
                            [128, 512], F32, tag="at", bufs=2, name=f"yp{hl}"
                        )
                        for j0 in range(8):
                            nc.tensor.matmul(
                                yp,
                                attPs[(hl, ci)][:, :, j0],
                                woP[:, j0, osl],
                                start=(j0 == 0),
                                stop=False,
                            )
                        nc.tensor.matmul(
                            yp,
                            ones_sb[0:1, 0:128],
                            bo_sb[0:1, hl, osl],
                            start=False,
                            stop=True,
                        )
                        nc.vector.tensor_copy(out=ysb[:, osl], in_=yp)
                    nc.sync.dma_start(out=y[hl, cb, :, :], in_=ysb)

            def emit_norm(ni, nsq, nat):
                # deferred one sq behind the tile loop so every sem wait
                # in this cross-engine chain is pre-satisfied
                nssl = slice(nsq * 512, (nsq + 1) * 512)
                for hl in range(2):
                    u = attTs[(hl, ni)]
                    nc.vector.tensor_copy(out=u[:, nssl], in_=nat[hl])
                    # fp16 reciprocal of the raw denominator row, in place
                    with nc.allow_low_precision(reason="fp16 recip of ~3e3"):
                        nc.vector.reciprocal(out=u[64:65, nssl], in_=u[64:65, nssl])
                    # broadcast 1/denom to all 128 partitions (gpsimd ucode)
                    bcs = smallp.tile([128, 512], AD, tag="bcs")
                    nc.gpsimd.partition_broadcast(bcs, u[64:65, nssl])
                    # odd s2 columns shifted to partitions 64:128
                    odd = ps.tile([128, 512], F32, tag="at", bufs=2, name="odd")
                    nc.tensor.matmul(
                        odd[64:128, 0:256],
                        eye_sb[0:64, 0:64],
                        u[0:64, nsq * 512 + 1 : (nsq + 1) * 512 : 2],
                        start=True, stop=True,
                    )
                    # normalize + repack into attP [p, rr, j0]
                    pa = attPs[(hl, ni)]
                    rsl = slice(32 * nsq, 32 * nsq + 32)
                    nc.gpsimd.tensor_mul(
                        pa[0:64, rsl, :],
                        u[0:64, nsq * 512 : (nsq + 1) * 512 : 2],
                        bcs[0:64, 0::2],
                    )
                    nc.vector.tensor_mul(
                        pa[64:128, rsl, :],
                        odd[64:128, 0:256],
                        bcs[64:128, 1::2],
                    )

            def load_acts(i):
                b = i % B
                tiles = {}
                # k first (earliest injection deadline), then q; v rides
                # the scalar-engine DGE queue concurrently
                for name in ("k", "q", "v"):
                    for piece in range(4):
                        psl = slice(piece * 512, (piece + 1) * 512)
                        a = actsp.tile(
                            [128, KT, 512], AD, tag="a" + name, name="a" + name
                        )
                        dma_eng[name].dma_start(
                            out=a, in_=dram_of[name][b, :, :, psl]
                        )
                        tiles[(name, piece)] = a
                return tiles

            projs = {}  # i -> {"q": qt, "k": kt, "v": vt}
            vnats = {}  # i -> vnat tile

            def emit_A_piece(ai, name, piece):
                # one 512-token piece of one projection of pair ai
                if ai not in projs:
                    projs[ai] = {}
                if name not in projs[ai]:
                    projs[ai][name] = projp.tile(
                        [128, S], AD, tag=name + "t", name=name + "t"
                    )
                dst = projs[ai][name]
                psl = slice(piece * 512, (piece + 1) * 512)
                a = acts_tiles[ai][(name, piece)]
                pj = ps.tile([128, 512], F32, tag="pj", bufs=2, name="pj")
                for k in range(KT):
                    nc.tensor.matmul(
                        pj,
                        w_sb[name][:, k, :],
                        a[:, k, :],
                        start=(k == 0),
                        stop=(k == KT - 1),
                    )
                if name == "q":
                    nc.vector.tensor_scalar_add(dst[:, psl], pj, bq_sb)
                else:
                    nc.vector.tensor_copy(out=dst[:, psl], in_=pj)

            def emit_Aprime_ones(ai):
                # v -> natural [t, v] layout, head-interleaved, with ones cols
                vnat = vaugp.tile([128, TT, 130], AD, tag="vnat", name="vnat")
                vnats[ai] = vnat
                nc.gpsimd.tensor_copy(
                    out=vnat[:, :, 128:130], in_=ones_sb[:, 0 : 2 * TT].rearrange(
                        "p (t two) -> p t two", two=2
                    )
                )

            def emit_Aprime_tr(ai, t0, cnt=4):
                vnat = vnats[ai]
                for tt in range(t0, t0 + cnt):
                    tp = ps.tile([128, 128], AD, tag="pj", bufs=2, name="tp")
                    nc.tensor.transpose(
                        tp,
                        projs[ai]["v"][:, tt * 128 : tt * 128 + 128],
                        eye_sb,
                    )
                    # psum cols (h, i) h-major -> sbuf col 2i+h
                    nc.vector.tensor_copy(
                        out=vnat[:, tt, 0:128]
                        .rearrange("p (i h) -> p i h", h=2)
                        .transpose([0, 2, 1]),
                        in_=tp,
                    )

            def emit_Aprime(ai):
                emit_Aprime_ones(ai)
                for t0 in range(0, TT, 4):
                    emit_Aprime_tr(ai, t0)

            acts_tiles = {0: load_acts(0)}
            # pair 0's A/A' run unhidden (nothing to overlap them with)
            for name in ("q", "k", "v"):
                for piece in range(4):
                    emit_A_piece(0, name, piece)
            emit_Aprime(0)

            # A(i+1)/A'(i+1) work injected into B(i)'s tile loop, keyed by
            # global slot g = sq*16 + tt (fires right after that tt's
            # score/exp emission). k first (earliest DMA + earliest need),
            # then q, then v; transposes after v's psum->sbuf copies.
            def make_inject_plan(ai):
                plan = {}
                for p in range(4):
                    plan[19 + 4 * p] = lambda p=p: emit_A_piece(ai, "k", p)
                    plan[35 + 4 * p] = lambda p=p: emit_A_piece(ai, "q", p)
                    plan[49 + 2 * p] = lambda p=p: emit_A_piece(ai, "v", p)
                plan[56] = lambda: emit_Aprime_ones(ai)
                for t0 in range(0, TT, 4):
                    plan[57 + t0 // 2] = lambda t0=t0: emit_Aprime_tr(ai, t0)
                return plan

            for i in range(npairs):
                for hl in range(2):
                    attTs[(hl, i)] = attp.tile([65, S], AD, tag="attT", name=f"attT{hl}")
                    attPs[(hl, i)] = attpk.tile(
                        [128, 128, 8], AD, tag="attP", name=f"attP{hl}"
                    )

                # prefetch next pair's activations during B(i) (also
                # across the rep boundary)
                if i + 1 < npairs:
                    acts_tiles[i + 1] = load_acts(i + 1)

                # ---------------- phase B: attention ----------------
                # (phase C of pair i-1 is emitted at sq 2/3 so its psum
                #  allocations FIFO-order behind B(i)'s first sc tiles)
                qt, kt = projs[i]["q"], projs[i]["k"]
                vnat = vnats[i]
                plan = make_inject_plan(i + 1) if i + 1 < npairs else {}
                norm_pend = []
                for sq in range(SQ):
                    if i > 0 and sq in (2, 3):
                        emit_C(i - 1, hls=(sq - 2,))
                    ssl = slice(sq * 512, (sq + 1) * 512)
                    at = [
                        ps.tile([65, 512], F32, tag="at", bufs=2, name=f"at{j}")
                        for j in range(2)
                    ]

                    def emit_av(ett, eex):
                        for hl in range(2):
                            nc.tensor.matmul(
                                at[hl],
                                vnat[:, ett, hl::2],
                                eex[:, hl * 512 : hl * 512 + 512],
                                start=(ett == 0),
                                stop=(ett == TT - 1),
                            )

                    pend = []  # AV lags exp by AVLAG tiles so its sem
                    # waits are pre-satisfied when PE reaches them
                    AVLAG = 4
                    for tt in range(TT):
                        tsl = slice(tt * 128, (tt + 1) * 128)
                        sc = ps.tile([128, 1024], F32, tag="sc", bufs=2, name="sc")
                        nc.tensor.matmul(
                            sc[:, 0:512], kt[0:64, tsl], qt[0:64, ssl],
                            start=True, stop=True,
                        )
                        nc.tensor.matmul(
                            sc[:, 512:1024], kt[64:128, tsl], qt[64:128, ssl],
                            start=True, stop=True,
                        )
                        ex = expp.tile([128, 1024], AD, tag="ex")
                        nc.scalar.activation(out=ex, in_=sc, func=AF.Exp, scale=0.125)
                        pend.append((tt, ex))
                        if len(pend) > AVLAG:
                            emit_av(*pend.pop(0))
                        g = sq * 16 + tt
                        if g in plan:
                            plan[g]()
                    for p in pend:
                        emit_av(*p)
                    norm_pend.append((sq, at))
                    if len(norm_pend) > 1:
                        emit_norm(i, *norm_pend.pop(0))
                for np_ in norm_pend:
                    emit_norm(i, *np_)

            emit_C(npairs - 1)

    nc.compile()
    return nc


def prep_inputs(query, key_, value, Wq, bq, Wk, bk, Wv, bv, Wo, bo):
    """Host-side sharding/packing. Returns in_maps for the 8 cores."""
    f32 = np.float32

    def packT(x):
        # [B, S, D] -> [B, 128, KT, S] fp16 (partition-major transposed)
        xt = np.asarray(x, f32).transpose(0, 2, 1).astype(NP_AD)  # [B, D, S]
        return np.ascontiguousarray(
            xt.reshape(B, KT, 128, S).transpose(0, 2, 1, 3)
        )

    qTn, kTn, vTn = packT(query), packT(key_), packT(value)
    Wq, Wk, Wv = (np.asarray(x, f32) for x in (Wq, Wk, Wv))
    bq, bv, Wo, bo = (np.asarray(x, f32) for x in (bq, bv, Wo, bo))
    wo_r = np.ascontiguousarray(
        Wo.reshape(8, 128, 1024).transpose(1, 0, 2).astype(NP_AD)
    )
    eye = np.eye(128, dtype=NP_AD)
    ones = np.ones((128, 128), dtype=NP_AD)
    in_maps = []
    for c in range(NCORES):
        h0, h1 = 2 * c, 2 * c + 1

        def pack(w):
            return np.ascontiguousarray(
                np.concatenate([w[h0], w[h1]], axis=1)
                .reshape(KT, 128, 128)
                .transpose(1, 0, 2)
                .astype(NP_AD)
            )

        bq_p = np.ascontiguousarray(
            np.concatenate([bq[h0], bq[h1]]).reshape(128, 1).astype(f32)
        )
        bo2 = np.ascontiguousarray(
            np.stack(
                [bo + np.tile(bv[h], H) @ Wo for h in (h0, h1)]
            ).astype(NP_AD)
        )
        in_maps.append(
            dict(
                qT=qTn, kT=kTn, vT=vTn,
                wq=pack(Wq), wk=pack(Wk), wv=pack(Wv),
                bq=bq_p, wo=wo_r, bo2=bo2, eye=eye, ones=ones,
            )
        )
    return in_maps


def assemble_output(results):
    out = np.empty((H * 256, 1024), np.float32)
    for c in range(NCORES):
        yc = results[c]["y"]  # [2, B, 128, 1024]
        for hl in range(2):
            h = 2 * c + hl
            out[h * 256 : (h + 1) * 256] = yc[hl].reshape(256, 1024)
    return out.reshape(B, S, D)


_NC_CACHE = {}
_CALLABLE_CACHE = {}


def _build_callable(nc):
    """Jit the bass module once (no donation; kernel writes all of y)."""
    import jax
    from jax.sharding import Mesh, PartitionSpec
    from jax.experimental.shard_map import shard_map
    from concourse import bass2jax
    import concourse.mybir as mb

    bass2jax.install_neuronx_cc_hook()
    pname = nc.partition_id_tensor.name if nc.partition_id_tensor else None
    in_names, out_names, out_avals = [], [], []
    for alloc in nc.m.functions[0].allocations:
        if not isinstance(alloc, mb.MemoryLocationSet):
            continue
        name = alloc.memorylocations[0].name
        if alloc.kind == "ExternalInput":
            if name != pname:
                in_names.append(name)
        elif alloc.kind == "ExternalOutput":
            out_names.append(name)
            out_avals.append(jax.core.ShapedArray(
                tuple(alloc.tensor_shape), mb.dt.np(alloc.dtype)))
    all_in = list(in_names) + list(out_names) + ([pname] if pname else [])
    zero_outs = [np.zeros(a.shape, a.dtype) for a in out_avals]

    def _body(*args):
        operands = list(args)
        if pname is not None:
            operands.append(bass2jax.partition_id_tensor())
        return tuple(bass2jax._bass_exec_p.bind(
            *operands, out_avals=tuple(out_avals), in_names=tuple(all_in),
            out_names=tuple(out_names), lowering_input_output_aliases=(),
            sim_require_finite=True, sim_require_nnan=True, nc=nc))

    mesh = Mesh(np.asarray(jax.devices()[:NCORES]), ("core",))
    # qT/kT/vT/eye/ones/wo are identical on every core: replicate (one
    # host->device transfer) instead of concatenating 8 copies
    shared = {"qT", "kT", "vT", "eye", "ones", "wo"}
    specs_in = tuple(
        PartitionSpec() if n in shared else PartitionSpec("core")
        for n in in_names
    ) + (PartitionSpec("core"),) * len(out_names)
    f = jax.jit(
        shard_map(_body, mesh=mesh, in_specs=specs_in,
                  out_specs=(PartitionSpec("core"),) * len(out_names),
                  check_rep=False),
        keep_unused=True,
    )
    return f, in_names, out_names, zero_outs, shared


def run(inputs, trace=False, reps=1):
    if reps not in _NC_CACHE:
        _NC_CACHE[reps] = build_nc(reps)
    nc = _NC_CACHE[reps]
    in_maps = prep_inputs(**inputs)
    try:
        import jax

        if reps not in _CALLABLE_CACHE:
            _CALLABLE_CACHE[reps] = _build_callable(nc)
        f, in_names, out_names, zero_outs, shared = _CALLABLE_CACHE[reps]
        concat_in = [
            np.asarray(in_maps[0][n]) if n in shared
            else np.concatenate([np.asarray(in_maps[c][n]) for c in range(NCORES)], 0)
            for n in in_names
        ] + [np.zeros((NCORES * z.shape[0], *z.shape[1:]), z.dtype)
             for z in zero_outs]
        out_arrs = [np.asarray(a) for a in f(*concat_in)]
        per_core = [
            a.reshape(NCORES, a.shape[0] // NCORES, *a.shape[1:])
            for a in out_arrs
        ]
        results = [
            {n: per_core[i][c] for i, n in enumerate(out_names)}
            for c in range(NCORES)
        ]

        class R:
            exec_time_ns = None
        return assemble_output(results), R()
    except Exception:
        from concourse.bass_utils import run_bass_kernel_spmd

        r = run_bass_kernel_spmd(nc, in_maps, list(range(NCORES)), trace=trace)
        return assemble_output(r.results), r


def kernel(**inputs) -> np.ndarray:
    out, _ = run(inputs, trace=False)
    return out


# revision 11
# speedup vs baseline: 1.2231x; 1.2231x over previous
"""MultiHeadAttention (B=2, S=2048, D=1024, H=16, dk=dv=64) on 8 trn2 cores.

Head-parallel: core c owns heads (2c, 2c+1). The reference's odd
reshape(B,-1,H*DV) means output row m draws only from head m//256, so the
final fc is fully local per core; host just concatenates.

Math transformations (exact, softmax-invariant):
  - bk dropped: adds a per-query constant to scores -> softmax unchanged.
  - bv folded into the output bias: softmax rows sum to 1, so
    att = w@v0 + bv; downstream y += tile16(bv) @ Wo, precomputed on host.
  - softmax without max-subtraction: scores ~ N(0,1), exp is safe in fp32.

v2 device dataflow per core (all fp16 streams, f32 psum):
  A  (per b): qT/kT [128=2x64 feats, 2048] = W.T @ actT (+bq for q).
  A' (per b): v -> natural [t, v] via [128,128] PE transposes; stored
      head-interleaved in vnat[:, tt, 0:128] (v_h at col 2i+h) with ones
      at cols 128,129 so lhsT = vnat[:, tt, h::2] is [t, 65] = {v_h|1}.
  B  (per b, per 512-col s-quarter): per t-tile: scoresT for both heads
      into one [128,1024] psum; one Exp ACT op; at_h[65,512] += vnat.T@ex
      (row 64 = denominator). Then per head: at->u copy, denominator
      broadcast via ones-matmul, fused reciprocal+copy -> bcs[128,512],
      odd-column partition-shift via eye-matmul, and two DVE multiplies
      that normalize AND repack into attP[128, rr, j0] (full-contract
      layout for the fc: rows 0:64 = even s2 = j even, 64:128 = odd).
  C  (per b): y[rr, o] = sum_j0 attP[:, :, j0].T @ woP[:, j0, o] (contract
      128 = two j's at once) + bias-mm; DMA straight from psum to DRAM.
"""

import numpy as np

import concourse.bacc as bacc
import concourse.mybir as mybir
import concourse.tile as tile

B, S, D, H, DK = 2, 2048, 1024, 16, 64
NCORES = 8
KT = D // 128  # 8 contraction tiles
TT = S // 128  # 16 t-tiles
SQ = S // 512  # 4 s-quarters
F32R = mybir.dt.float32r
F32 = mybir.dt.float32
F16 = mybir.dt.float16
AD = F16
NP_AD = np.float16
AF = mybir.ActivationFunctionType


def build_nc(reps=1):
    nc = bacc.Bacc(trn_type="TRN2")

    qT = nc.declare_dram_parameter("qT", [B, 128, KT, S], AD, isOutput=False)
    kTd = nc.declare_dram_parameter("kT", [B, 128, KT, S], AD, isOutput=False)
    vTd = nc.declare_dram_parameter("vT", [B, 128, KT, S], AD, isOutput=False)
    wq = nc.declare_dram_parameter("wq", [128, KT, 128], AD, isOutput=False)
    wk = nc.declare_dram_parameter("wk", [128, KT, 128], AD, isOutput=False)
    wv = nc.declare_dram_parameter("wv", [128, KT, 128], AD, isOutput=False)
    bqd = nc.declare_dram_parameter("bq", [128, 1], F32, isOutput=False)
    eyed = nc.declare_dram_parameter("eye", [128, 128], AD, isOutput=False)
    onesd = nc.declare_dram_parameter("ones", [128, 128], AD, isOutput=False)
    wo = nc.declare_dram_parameter("wo", [128, 8, 1024], AD, isOutput=False)
    bo2 = nc.declare_dram_parameter("bo2", [2, 1024], AD, isOutput=False)
    y = nc.declare_dram_parameter("y", [2, B, 128, 1024], F16, isOutput=True)

    with tile.TileContext(nc) as tc:
        with (
            tc.tile_pool(name="const", bufs=1) as constp,
            tc.tile_pool(name="wts", bufs=1) as wtsp,
            tc.tile_pool(name="acts", bufs=4) as actsp,
            tc.tile_pool(name="proj", bufs=2) as projp,
            tc.tile_pool(name="vaugp", bufs=2) as vaugp,
            tc.tile_pool(name="exp", bufs=6) as expp,
            tc.tile_pool(name="attp", bufs=4) as attp,
            tc.tile_pool(name="attpk", bufs=4) as attpk,
            tc.tile_pool(name="small", bufs=3) as smallp,
            tc.tile_pool(name="wop", bufs=1) as wop,
            tc.tile_pool(name="ps", bufs=1, space="PSUM") as ps,
        ):
            # constants
            eye_sb = constp.tile([128, 128], AD, tag="eye")
            nc.sync.dma_start(out=eye_sb, in_=eyed[:, :])
            ones_sb = constp.tile([128, 128], AD, tag="ones_sb")
            nc.sync.dma_start(out=ones_sb, in_=onesd[:, :])
            bq_sb = constp.tile([128, 1], F32, tag="bq")
            nc.sync.dma_start(out=bq_sb, in_=bqd[:, :])
            bo_sb = constp.tile([1, 2, 1024], AD, tag="bo")
            nc.sync.dma_start(out=bo_sb, in_=bo2[None, :, :])

            # packed per-head weights, resident
            w_sb = {}
            for name, dram in (("q", wq), ("k", wk), ("v", wv)):
                w_sb[name] = wtsp.tile([128, KT, 128], AD, tag="w" + name, name="w" + name)
                nc.sync.dma_start(out=w_sb[name], in_=dram[:, :, :])

            # fc weights resident across reps (like wq/wk/wv)
            woP = wop.tile([128, 8, 1024], AD, tag="woP", name="woP")
            for j in range(2):
                nc.sync.dma_start(
                    out=woP[:, 4 * j : 4 * j + 4, :], in_=wo[:, 4 * j : 4 * j + 4, :]
                )

            for rep in range(reps):
                attTs = {}  # (hl, b) -> [65, S]: rows 0-63 unnormalized attT,
                # row 64 = raw softmax denominator
                attPs = {}  # (hl, b) -> [128, 128, 8] normalized+repacked
                for hl in range(2):
                    for b in range(B):
                        attTs[(hl, b)] = attp.tile([65, S], AD, tag="attT", name=f"attT{hl}{b}")
                        attPs[(hl, b)] = attpk.tile(
                            [128, 128, 8], AD, tag="attP", name=f"attP{hl}{b}"
                        )

                dram_of = {"q": qT, "k": kTd, "v": vTd}
                dma_eng = {"q": nc.gpsimd, "k": nc.sync, "v": nc.gpsimd}

                def emit_C(cb, hls=(0, 1)):
                    # ---------------- phase C: output fc ----------------
                    for hl in hls:
                        ysb = smallp.tile([128, 1024], F16, tag="ysb", name=f"ysb{hl}")
                        for ob in range(2):
                            osl = slice(ob * 512, (ob + 1) * 512)
                            yp = ps.tile(
                                [128, 512], F32, tag="at", bufs=2, name=f"yp{hl}"
                            )
                            for j0 in range(8):
                                nc.tensor.matmul(
                                    yp,
                                    attPs[(hl, cb)][:, :, j0],
                                    woP[:, j0, osl],
                                    start=(j0 == 0),
                                    stop=False,
                                )
                            nc.tensor.matmul(
                                yp,
                                ones_sb[0:1, 0:128],
                                bo_sb[0:1, hl, osl],
                                start=False,
                                stop=True,
                            )
                            nc.vector.tensor_copy(out=ysb[:, osl], in_=yp)
                        nc.sync.dma_start(out=y[hl, cb, :, :], in_=ysb)

                def emit_norm(nb, nsq, nat):
                    # deferred one sq behind the tile loop so every sem wait
                    # in this 5-step cross-engine chain is pre-satisfied
                    nssl = slice(nsq * 512, (nsq + 1) * 512)
                    for hl in range(2):
                        u = attTs[(hl, nb)]
                        nc.vector.tensor_copy(out=u[:, nssl], in_=nat[hl])
                        # broadcast raw denominator to all 128 partitions
                        bc = ps.tile([128, 512], F32, tag="at", bufs=2, name="bc")
                        nc.tensor.matmul(
                            bc, ones_sb[64:65, 0:128], u[64:65, nssl],
                            start=True, stop=True,
                        )
                        bcs = smallp.tile([128, 512], AD, tag="bcs")
                        with nc.allow_low_precision(reason="fp16 recip of ~3e3"):
                            nc.vector.reciprocal(out=bcs, in_=bc)
                        # odd s2 columns shifted to partitions 64:128
                        odd = ps.tile([128, 512], F32, tag="at", bufs=2, name="odd")
                        nc.tensor.matmul(
                            odd[64:128, 0:256],
                            eye_sb[0:64, 0:64],
                            u[0:64, nsq * 512 + 1 : (nsq + 1) * 512 : 2],
                            start=True, stop=True,
                        )
                        # normalize + repack into attP [p, rr, j0]
                        pa = attPs[(hl, nb)]
                        rsl = slice(32 * nsq, 32 * nsq + 32)
                        nc.gpsimd.tensor_mul(
                            pa[0:64, rsl, :],
                            u[0:64, nsq * 512 : (nsq + 1) * 512 : 2],
                            bcs[0:64, 0::2],
                        )
                        nc.vector.tensor_mul(
                            pa[64:128, rsl, :],
                            odd[64:128, 0:256],
                            bcs[64:128, 1::2],
                        )

                def load_acts(b):
                    tiles = {}
                    # q/k first (gate scores); v deferred (only needed by AV)
                    for name in ("q", "k", "v"):
                        for piece in range(4):
                            psl = slice(piece * 512, (piece + 1) * 512)
                            a = actsp.tile(
                                [128, KT, 512], AD, tag="a" + name, name="a" + name
                            )
                            dma_eng[name].dma_start(
                                out=a, in_=dram_of[name][b, :, :, psl]
                            )
                            tiles[(name, piece)] = a
                    return tiles

                acts_tiles = {0: load_acts(0)}

                for b in range(B):
                    # ---------------- phase A: projections ----------------
                    proj = {}
                    for name in ("q", "k", "v"):
                        dst = projp.tile([128, S], AD, tag=name + "t", name=name + "t")
                        for piece in range(4):
                            psl = slice(piece * 512, (piece + 1) * 512)
                            a = acts_tiles[b][(name, piece)]
                            pj = ps.tile([128, 512], F32, tag="pj", bufs=2, name="pj")
                            for k in range(KT):
                                nc.tensor.matmul(
                                    pj,
                                    w_sb[name][:, k, :],
                                    a[:, k, :],
                                    start=(k == 0),
                                    stop=(k == KT - 1),
                                )
                            if name == "q":
                                nc.vector.tensor_scalar_add(dst[:, psl], pj, bq_sb)
                            else:
                                nc.vector.tensor_copy(out=dst[:, psl], in_=pj)
                        proj[name] = dst

                    # ------- phase A': v -> natural, head-interleaved ------
                    vnat = vaugp.tile([128, TT, 130], AD, tag="vnat", name="vnat")
                    nc.gpsimd.tensor_copy(
                        out=vnat[:, :, 128:130], in_=ones_sb[:, 0 : 2 * TT].rearrange(
                            "p (t two) -> p t two", two=2
                        )
                    )
                    for tt in range(TT):
                        tp = ps.tile([128, 128], AD, tag="pj", bufs=2, name="tp")
                        nc.tensor.transpose(
                            tp,
                            proj["v"][:, tt * 128 : tt * 128 + 128],
                            eye_sb,
                        )
                        # psum cols (h, i) h-major -> sbuf col 2i+h
                        nc.vector.tensor_copy(
                            out=vnat[:, tt, 0:128]
                            .rearrange("p (i h) -> p i h", h=2)
                            .transpose([0, 2, 1]),
                            in_=tp,
                        )

                    # prefetch next batch's activations during B(b)
                    if b + 1 < B:
                        acts_tiles[b + 1] = load_acts(b + 1)

                    # ---------------- phase B: attention ----------------
                    # (phase C of batch b-1 is emitted after sq0 so its psum
                    #  allocations FIFO-order behind B(b)'s first sc tiles)
                    qt, kt = proj["q"], proj["k"]
                    norm_pend = []
                    for sq in range(SQ):
                        if b > 0 and sq in (2, 3):
                            emit_C(b - 1, hls=(sq - 2,))
                        ssl = slice(sq * 512, (sq + 1) * 512)
                        at = [
                            ps.tile([65, 512], F32, tag="at", bufs=2, name=f"at{i}")
                            for i in range(2)
                        ]
                        def emit_av(ett, eex):
                            for hl in range(2):
                                nc.tensor.matmul(
                                    at[hl],
                                    vnat[:, ett, hl::2],
                                    eex[:, hl * 512 : hl * 512 + 512],
                                    start=(ett == 0),
                                    stop=(ett == TT - 1),
                                )

                        pend = []  # AV lags exp by AVLAG tiles so its sem
                        # waits are pre-satisfied when PE reaches them
                        AVLAG = 4
                        for tt in range(TT):
                            tsl = slice(tt * 128, (tt + 1) * 128)
                            sc = ps.tile([128, 1024], F32, tag="sc", bufs=2, name="sc")
                            nc.tensor.matmul(
                                sc[:, 0:512], kt[0:64, tsl], qt[0:64, ssl],
                                start=True, stop=True,
                            )
                            nc.tensor.matmul(
                                sc[:, 512:1024], kt[64:128, tsl], qt[64:128, ssl],
                                start=True, stop=True,
                            )
                            ex = expp.tile([128, 1024], AD, tag="ex")
                            nc.scalar.activation(out=ex, in_=sc, func=AF.Exp, scale=0.125)
                            pend.append((tt, ex))
                            if len(pend) > AVLAG:
                                emit_av(*pend.pop(0))
                        for p in pend:
                            emit_av(*p)
                        norm_pend.append((sq, at))
                        if len(norm_pend) > 1:
                            emit_norm(b, *norm_pend.pop(0))
                    for np_ in norm_pend:
                        emit_norm(b, *np_)
                    norm_pend = []

                emit_C(B - 1)

    nc.compile()
    return nc


def prep_inputs(query, key_, value, Wq, bq, Wk, bk, Wv, bv, Wo, bo):
    """Host-side sharding/packing. Returns in_maps for the 8 cores."""
    f32 = np.float32

    def packT(x):
        # [B, S, D] -> [B, 128, KT, S] fp16 (partition-major transposed)
        xt = np.asarray(x, f32).transpose(0, 2, 1).astype(NP_AD)  # [B, D, S]
        return np.ascontiguousarray(
            xt.reshape(B, KT, 128, S).transpose(0, 2, 1, 3)
        )

    qTn, kTn, vTn = packT(query), packT(key_), packT(value)
    Wq, Wk, Wv = (np.asarray(x, f32) for x in (Wq, Wk, Wv))
    bq, bv, Wo, bo = (np.asarray(x, f32) for x in (bq, bv, Wo, bo))
    wo_r = np.ascontiguousarray(
        Wo.reshape(8, 128, 1024).transpose(1, 0, 2).astype(NP_AD)
    )
    eye = np.eye(128, dtype=NP_AD)
    ones = np.ones((128, 128), dtype=NP_AD)
    in_maps = []
    for c in range(NCORES):
        h0, h1 = 2 * c, 2 * c + 1

        def pack(w):
            return np.ascontiguousarray(
                np.concatenate([w[h0], w[h1]], axis=1)
                .reshape(KT, 128, 128)
                .transpose(1, 0, 2)
                .astype(NP_AD)
            )

        bq_p = np.ascontiguousarray(
            np.concatenate([bq[h0], bq[h1]]).reshape(128, 1).astype(f32)
        )
        bo2 = np.ascontiguousarray(
            np.stack(
                [bo + np.tile(bv[h], H) @ Wo for h in (h0, h1)]
            ).astype(NP_AD)
        )
        in_maps.append(
            dict(
                qT=qTn, kT=kTn, vT=vTn,
                wq=pack(Wq), wk=pack(Wk), wv=pack(Wv),
                bq=bq_p, wo=wo_r, bo2=bo2, eye=eye, ones=ones,
            )
        )
    return in_maps


def assemble_output(results):
    out = np.empty((H * 256, 1024), np.float32)
    for c in range(NCORES):
        yc = results[c]["y"]  # [2, B, 128, 1024]
        for hl in range(2):
            h = 2 * c + hl
            out[h * 256 : (h + 1) * 256] = yc[hl].reshape(256, 1024)
    return out.reshape(B, S, D)


_NC_CACHE = {}
_CALLABLE_CACHE = {}


def _build_callable(nc):
    """Jit the bass module once (no donation; kernel writes all of y)."""
    import jax
    from jax.sharding import Mesh, PartitionSpec
    from jax.experimental.shard_map import shard_map
    from concourse import bass2jax
    import concourse.mybir as mb

    bass2jax.install_neuronx_cc_hook()
    pname = nc.partition_id_tensor.name if nc.partition_id_tensor else None
    in_names, out_names, out_avals = [], [], []
    for alloc in nc.m.functions[0].allocations:
        if not isinstance(alloc, mb.MemoryLocationSet):
            continue
        name = alloc.memorylocations[0].name
        if alloc.kind == "ExternalInput":
            if name != pname:
                in_names.append(name)
        elif alloc.kind == "ExternalOutput":
            out_names.append(name)
            out_avals.append(jax.core.ShapedArray(
                tuple(alloc.tensor_shape), mb.dt.np(alloc.dtype)))
    all_in = list(in_names) + list(out_names) + ([pname] if pname else [])
    zero_outs = [np.zeros(a.shape, a.dtype) for a in out_avals]

    def _body(*args):
        operands = list(args)
        if pname is not None:
            operands.append(bass2jax.partition_id_tensor())
        return tuple(bass2jax._bass_exec_p.bind(
            *operands, out_avals=tuple(out_avals), in_names=tuple(all_in),
            out_names=tuple(out_names), lowering_input_output_aliases=(),
            sim_require_finite=True, sim_require_nnan=True, nc=nc))

    mesh = Mesh(np.asarray(jax.devices()[:NCORES]), ("core",))
    # qT/kT/vT/eye/ones/wo are identical on every core: replicate (one
    # host->device transfer) instead of concatenating 8 copies
    shared = {"qT", "kT", "vT", "eye", "ones", "wo"}
    specs_in = tuple(
        PartitionSpec() if n in shared else PartitionSpec("core")
        for n in in_names
    ) + (PartitionSpec("core"),) * len(out_names)
    f = jax.jit(
        shard_map(_body, mesh=mesh, in_specs=specs_in,
                  out_specs=(PartitionSpec("core"),) * len(out_names),
                  check_rep=False),
        keep_unused=True,
    )
    return f, in_names, out_names, zero_outs, shared


def run(inputs, trace=False, reps=1):
    if reps not in _NC_CACHE:
        _NC_CACHE[reps] = build_nc(reps)
    nc = _NC_CACHE[reps]
    in_maps = prep_inputs(**inputs)
    try:
        import jax

        if reps not in _CALLABLE_CACHE:
            _CALLABLE_CACHE[reps] = _build_callable(nc)
        f, in_names, out_names, zero_outs, shared = _CALLABLE_CACHE[reps]
        concat_in = [
            np.asarray(in_maps[0][n]) if n in shared
            else np.concatenate([np.asarray(in_maps[c][n]) for c in range(NCORES)], 0)
            for n in in_names
        ] + [np.zeros((NCORES * z.shape[0], *z.shape[1:]), z.dtype)
             for z in zero_outs]
        out_arrs = [np.asarray(a) for a in f(*concat_in)]
        per_core = [
            a.reshape(NCORES, a.shape[0] // NCORES, *a.shape[1:])
            for a in out_arrs
        ]
        results = [
            {n: per_core[i][c] for i, n in enumerate(out_names)}
            for c in range(NCORES)
        ]

        class R:
            exec_time_ns = None
        return assemble_output(results), R()
    except Exception:
        from concourse.bass_utils import run_bass_kernel_spmd

        r = run_bass_kernel_spmd(nc, in_maps, list(range(NCORES)), trace=trace)
        return assemble_output(r.results), r


def kernel(**inputs) -> np.ndarray:
    out, _ = run(inputs, trace=False)
    return out


# revision 13
# speedup vs baseline: 1.3075x; 1.0690x over previous
"""MultiHeadAttention (B=2, S=2048, D=1024, H=16, dk=dv=64) on 8 trn2 cores.

Head-parallel: core c owns heads (2c, 2c+1). The reference's odd
reshape(B,-1,H*DV) means output row m draws only from head m//256, so the
final fc is fully local per core; host just concatenates.

Math transformations (exact, softmax-invariant):
  - bk dropped: adds a per-query constant to scores -> softmax unchanged.
  - bv folded into the output bias: softmax rows sum to 1, so
    att = w@v0 + bv; downstream y += tile16(bv) @ Wo, precomputed on host.
  - softmax without max-subtraction: scores ~ N(0,1), exp is safe in fp32.

v3 device dataflow per core (all fp16 streams, f32 psum):
  A  (per pair): qT/kT [128=2x64 feats, 2048] = W.T @ actT (+bq for q).
  A' (per pair): v -> natural [t, v] via [128,128] PE transposes; stored
      head-interleaved in vnat[:, tt, 0:128] (v_h at col 2i+h) with ones
      at cols 128,129 so lhsT = vnat[:, tt, h::2] is [t, 65] = {v_h|1}.
  B  (per pair, per 512-col s-quarter): per t-tile: scoresT for both heads
      into one [128,1024] psum (two K=64 matmuls at tile_position rows 0/64
      -> HW-concurrent); one Exp ACT op; at_h[65,512] += vnat.T@ex
      (row 64 = raw softmax denominator). Then per head: at->u copy,
      denominator broadcast via ones-matmul, fused reciprocal -> bcs,
      odd-column partition-shift via eye-matmul, and two multiplies
      (gpsimd even / DVE odd) that normalize AND repack into
      attP[128, rr, j0] (full-contract layout for the fc: rows 0:64 =
      even s2 = j even, 64:128 = odd).
  C  (per pair): y[rr, o] = sum_j0 attP[:, :, j0].T @ woP[:, j0, o]
      (contract 128 = two j's at once) + bias-mm; DMA from psum to DRAM.
  Pipelining: activations for pair i+1 prefetch during B(i) (works across
  the rep boundary too); emit_C(i-1) is interleaved into B(i) at sq 2/3.
"""

import numpy as np

import concourse.bacc as bacc
import concourse.mybir as mybir
import concourse.tile as tile

B, S, D, H, DK = 2, 2048, 1024, 16, 64
NCORES = 8
KT = D // 128  # 8 contraction tiles
TT = S // 128  # 16 t-tiles
SQ = S // 512  # 4 s-quarters
F32R = mybir.dt.float32r
F32 = mybir.dt.float32
F16 = mybir.dt.float16
AD = F16
NP_AD = np.float16
AF = mybir.ActivationFunctionType


def build_nc(reps=1):
    nc = bacc.Bacc(trn_type="TRN2")

    qT = nc.declare_dram_parameter("qT", [B, 128, KT, S], AD, isOutput=False)
    kTd = nc.declare_dram_parameter("kT", [B, 128, KT, S], AD, isOutput=False)
    vTd = nc.declare_dram_parameter("vT", [B, 128, KT, S], AD, isOutput=False)
    wq = nc.declare_dram_parameter("wq", [128, KT, 128], AD, isOutput=False)
    wk = nc.declare_dram_parameter("wk", [128, KT, 128], AD, isOutput=False)
    wv = nc.declare_dram_parameter("wv", [128, KT, 128], AD, isOutput=False)
    bqd = nc.declare_dram_parameter("bq", [128, 1], F32, isOutput=False)
    eyed = nc.declare_dram_parameter("eye", [128, 128], AD, isOutput=False)
    onesd = nc.declare_dram_parameter("ones", [128, 128], AD, isOutput=False)
    wo = nc.declare_dram_parameter("wo", [128, 8, 1024], AD, isOutput=False)
    bo2 = nc.declare_dram_parameter("bo2", [2, 1024], AD, isOutput=False)
    y = nc.declare_dram_parameter("y", [2, B, 128, 1024], F16, isOutput=True)

    with tile.TileContext(nc) as tc:
        with (
            tc.tile_pool(name="const", bufs=1) as constp,
            tc.tile_pool(name="wts", bufs=1) as wtsp,
            tc.tile_pool(name="acts", bufs=4) as actsp,
            tc.tile_pool(name="proj", bufs=2) as projp,
            tc.tile_pool(name="vaugp", bufs=2) as vaugp,
            tc.tile_pool(name="exp", bufs=6) as expp,
            tc.tile_pool(name="attp", bufs=4) as attp,
            tc.tile_pool(name="attpk", bufs=4) as attpk,
            tc.tile_pool(name="small", bufs=3) as smallp,
            tc.tile_pool(name="wop", bufs=1) as wop,
            tc.tile_pool(name="ps", bufs=1, space="PSUM") as ps,
        ):
            # constants
            eye_sb = constp.tile([128, 128], AD, tag="eye")
            nc.sync.dma_start(out=eye_sb, in_=eyed[:, :])
            ones_sb = constp.tile([128, 128], AD, tag="ones_sb")
            nc.sync.dma_start(out=ones_sb, in_=onesd[:, :])
            bq_sb = constp.tile([128, 1], F32, tag="bq")
            nc.sync.dma_start(out=bq_sb, in_=bqd[:, :])
            bo_sb = constp.tile([1, 2, 1024], AD, tag="bo")
            nc.sync.dma_start(out=bo_sb, in_=bo2[None, :, :])

            # packed per-head weights, resident
            w_sb = {}
            for name, dram in (("q", wq), ("k", wk), ("v", wv)):
                w_sb[name] = wtsp.tile([128, KT, 128], AD, tag="w" + name, name="w" + name)
                nc.sync.dma_start(out=w_sb[name], in_=dram[:, :, :])

            # fc weights resident across reps (like wq/wk/wv)
            woP = wop.tile([128, 8, 1024], AD, tag="woP", name="woP")
            for j in range(2):
                nc.sync.dma_start(
                    out=woP[:, 4 * j : 4 * j + 4, :], in_=wo[:, 4 * j : 4 * j + 4, :]
                )

            npairs = reps * B
            attTs = {}  # (hl, i) -> [65, S]: rows 0-63 unnormalized attT,
            # row 64 = raw softmax denominator (reciprocal'd in place)
            attPs = {}  # (hl, i) -> [128, 128, 8] normalized+repacked

            dram_of = {"q": qT, "k": kTd, "v": vTd}
            dma_eng = {"q": nc.sync, "k": nc.sync, "v": nc.gpsimd}

            def emit_C(ci, hls=(0, 1)):
                # ---------------- phase C: output fc ----------------
                cb = ci % B
                for hl in hls:
                    ysb = smallp.tile([128, 1024], F16, tag="ysb", name=f"ysb{hl}")
                    for ob in range(2):
                        osl = slice(ob * 512, (ob + 1) * 512)
                        yp = ps.tile(
                            [128, 512], F32, tag="at", bufs=2, name=f"yp{hl}"
                        )
                        for j0 in range(8):
                            nc.tensor.matmul(
                                yp,
                                attPs[(hl, ci)][:, :, j0],
                                woP[:, j0, osl],
                                start=(j0 == 0),
                                stop=False,
                            )
                        nc.tensor.matmul(
                            yp,
                            ones_sb[0:1, 0:128],
                            bo_sb[0:1, hl, osl],
                            start=False,
                            stop=True,
                        )
                        nc.vector.tensor_copy(out=ysb[:, osl], in_=yp)
                    nc.sync.dma_start(out=y[hl, cb, :, :], in_=ysb)

            def emit_norm(ni, nsq, nat):
                # deferred one sq behind the tile loop so every sem wait
                # in this cross-engine chain is pre-satisfied
                nssl = slice(nsq * 512, (nsq + 1) * 512)
                for hl in range(2):
                    u = attTs[(hl, ni)]
                    nc.vector.tensor_copy(out=u[:, nssl], in_=nat[hl])
                    # broadcast raw denominator to all 128 partitions
                    bc = ps.tile([128, 512], F32, tag="at", bufs=2, name="bc")
                    nc.tensor.matmul(
                        bc, ones_sb[64:65, 0:128], u[64:65, nssl],
                        start=True, stop=True,
                    )
                    bcs = smallp.tile([128, 512], AD, tag="bcs")
                    with nc.allow_low_precision(reason="fp16 recip of ~3e3"):
                        nc.vector.reciprocal(out=bcs, in_=bc)
                    # odd s2 columns shifted to partitions 64:128
                    odd = ps.tile([128, 512], F32, tag="at", bufs=2, name="odd")
                    nc.tensor.matmul(
                        odd[64:128, 0:256],
                        eye_sb[0:64, 0:64],
                        u[0:64, nsq * 512 + 1 : (nsq + 1) * 512 : 2],
                        start=True, stop=True,
                    )
                    # normalize + repack into attP [p, rr, j0]
                    pa = attPs[(hl, ni)]
                    rsl = slice(32 * nsq, 32 * nsq + 32)
                    nc.gpsimd.tensor_mul(
                        pa[0:64, rsl, :],
                        u[0:64, nsq * 512 : (nsq + 1) * 512 : 2],
                        bcs[0:64, 0::2],
                    )
                    nc.vector.tensor_mul(
                        pa[64:128, rsl, :],
                        odd[64:128, 0:256],
                        bcs[64:128, 1::2],
                    )

            def load_acts(i):
                b = i % B
                tiles = {}
                # k first (earliest injection deadline), then q; v rides
                # the scalar-engine DGE queue concurrently
                for name in ("k", "q", "v"):
                    for piece in range(4):
                        psl = slice(piece * 512, (piece + 1) * 512)
                        a = actsp.tile(
                            [128, KT, 512], AD, tag="a" + name, name="a" + name
                        )
                        dma_eng[name].dma_start(
                            out=a, in_=dram_of[name][b, :, :, psl]
                        )
                        tiles[(name, piece)] = a
                return tiles

            projs = {}  # i -> {"q": qt, "k": kt, "v": vt}
            vnats = {}  # i -> vnat tile

            def emit_A_piece(ai, name, piece):
                # one 512-token piece of one projection of pair ai
                if ai not in projs:
                    projs[ai] = {}
                if name not in projs[ai]:
                    projs[ai][name] = projp.tile(
                        [128, S], AD, tag=name + "t", name=name + "t"
                    )
                dst = projs[ai][name]
                psl = slice(piece * 512, (piece + 1) * 512)
                a = acts_tiles[ai][(name, piece)]
                pj = ps.tile([128, 512], F32, tag="pj", bufs=2, name="pj")
                for k in range(KT):
                    nc.tensor.matmul(
                        pj,
                        w_sb[name][:, k, :],
                        a[:, k, :],
                        start=(k == 0),
                        stop=(k == KT - 1),
                    )
                if name == "q":
                    nc.vector.tensor_scalar_add(dst[:, psl], pj, bq_sb)
                else:
                    nc.vector.tensor_copy(out=dst[:, psl], in_=pj)

            def emit_Aprime_ones(ai):
                # v -> natural [t, v] layout, head-interleaved, with ones cols
                vnat = vaugp.tile([128, TT, 130], AD, tag="vnat", name="vnat")
                vnats[ai] = vnat
                nc.gpsimd.tensor_copy(
                    out=vnat[:, :, 128:130], in_=ones_sb[:, 0 : 2 * TT].rearrange(
                        "p (t two) -> p t two", two=2
                    )
                )

            def emit_Aprime_tr(ai, t0, cnt=4):
                vnat = vnats[ai]
                for tt in range(t0, t0 + cnt):
                    tp = ps.tile([128, 128], AD, tag="pj", bufs=2, name="tp")
                    nc.tensor.transpose(
                        tp,
                        projs[ai]["v"][:, tt * 128 : tt * 128 + 128],
                        eye_sb,
                    )
                    # psum cols (h, i) h-major -> sbuf col 2i+h
                    nc.vector.tensor_copy(
                        out=vnat[:, tt, 0:128]
                        .rearrange("p (i h) -> p i h", h=2)
                        .transpose([0, 2, 1]),
                        in_=tp,
                    )

            def emit_Aprime(ai):
                emit_Aprime_ones(ai)
                for t0 in range(0, TT, 4):
                    emit_Aprime_tr(ai, t0)

            acts_tiles = {0: load_acts(0)}
            # pair 0's A/A' run unhidden (nothing to overlap them with)
            for name in ("q", "k", "v"):
                for piece in range(4):
                    emit_A_piece(0, name, piece)
            emit_Aprime(0)

            # A(i+1)/A'(i+1) work injected into B(i)'s tile loop, keyed by
            # global slot g = sq*16 + tt (fires right after that tt's
            # score/exp emission). k first (earliest DMA + earliest need),
            # then q, then v; transposes after v's psum->sbuf copies.
            def make_inject_plan(ai):
                plan = {}
                for p in range(4):
                    plan[19 + 4 * p] = lambda p=p: emit_A_piece(ai, "k", p)
                    plan[35 + 4 * p] = lambda p=p: emit_A_piece(ai, "q", p)
                    plan[49 + 2 * p] = lambda p=p: emit_A_piece(ai, "v", p)
                plan[56] = lambda: emit_Aprime_ones(ai)
                for t0 in range(0, TT, 4):
                    plan[57 + t0 // 2] = lambda t0=t0: emit_Aprime_tr(ai, t0)
                return plan

            for i in range(npairs):
                for hl in range(2):
                    attTs[(hl, i)] = attp.tile([65, S], AD, tag="attT", name=f"attT{hl}")
                    attPs[(hl, i)] = attpk.tile(
                        [128, 128, 8], AD, tag="attP", name=f"attP{hl}"
                    )

                # prefetch next pair's activations during B(i) (also
                # across the rep boundary)
                if i + 1 < npairs:
                    acts_tiles[i + 1] = load_acts(i + 1)

                # ---------------- phase B: attention ----------------
                # (phase C of pair i-1 is emitted at sq 2/3 so its psum
                #  allocations FIFO-order behind B(i)'s first sc tiles)
                qt, kt = projs[i]["q"], projs[i]["k"]
                vnat = vnats[i]
                plan = make_inject_plan(i + 1) if i + 1 < npairs else {}
                norm_pend = []
                for sq in range(SQ):
                    if i > 0 and sq in (2, 3):
                        emit_C(i - 1, hls=(sq - 2,))
                    ssl = slice(sq * 512, (sq + 1) * 512)
                    at = [
                        ps.tile([65, 512], F32, tag="at", bufs=2, name=f"at{j}")
                        for j in range(2)
                    ]

                    def emit_av(ett, eex):
                        for hl in range(2):
                            nc.tensor.matmul(
                                at[hl],
                                vnat[:, ett, hl::2],
                                eex[:, hl * 512 : hl * 512 + 512],
                                start=(ett == 0),
                                stop=(ett == TT - 1),
                            )

                    pend = []  # AV lags exp by AVLAG tiles so its sem
                    # waits are pre-satisfied when PE reaches them
                    AVLAG = 4
                    for tt in range(TT):
                        tsl = slice(tt * 128, (tt + 1) * 128)
                        sc = ps.tile([128, 1024], F32, tag="sc", bufs=2, name="sc")
                        nc.tensor.matmul(
                            sc[:, 0:512], kt[0:64, tsl], qt[0:64, ssl],
                            start=True, stop=True,
                        )
                        nc.tensor.matmul(
                            sc[:, 512:1024], kt[64:128, tsl], qt[64:128, ssl],
                            start=True, stop=True,
                        )
                        ex = expp.tile([128, 1024], AD, tag="ex")
                        nc.scalar.activation(out=ex, in_=sc, func=AF.Exp, scale=0.125)
                        pend.append((tt, ex))
                        if len(pend) > AVLAG:
                            emit_av(*pend.pop(0))
                        g = sq * 16 + tt
                        if g in plan:
                            plan[g]()
                    for p in pend:
                        emit_av(*p)
                    norm_pend.append((sq, at))
                    if len(norm_pend) > 1:
                        emit_norm(i, *norm_pend.pop(0))
                for np_ in norm_pend:
                    emit_norm(i, *np_)

            emit_C(npairs - 1)

    nc.compile()
    return nc


def prep_inputs(query, key_, value, Wq, bq, Wk, bk, Wv, bv, Wo, bo):
    """Host-side sharding/packing. Returns in_maps for the 8 cores."""
    f32 = np.float32

    def packT(x):
        # [B, S, D] -> [B, 128, KT, S] fp16 (partition-major transposed)
        xt = np.asarray(x, f32).transpose(0, 2, 1).astype(NP_AD)  # [B, D, S]
        return np.ascontiguousarray(
            xt.reshape(B, KT, 128, S).transpose(0, 2, 1, 3)
        )

    qTn, kTn, vTn = packT(query), packT(key_), packT(value)
    Wq, Wk, Wv = (np.asarray(x, f32) for x in (Wq, Wk, Wv))
    bq, bv, Wo, bo = (np.asarray(x, f32) for x in (bq, bv, Wo, bo))
    wo_r = np.ascontiguousarray(
        Wo.reshape(8, 128, 1024).transpose(1, 0, 2).astype(NP_AD)
    )
    eye = np.eye(128, dtype=NP_AD)
    ones = np.ones((128, 128), dtype=NP_AD)
    in_maps = []
    for c in range(NCORES):
        h0, h1 = 2 * c, 2 * c + 1

        def pack(w):
            return np.ascontiguousarray(
                np.concatenate([w[h0], w[h1]], axis=1)
                .reshape(KT, 128, 128)
                .transpose(1, 0, 2)
                .astype(NP_AD)
            )

        bq_p = np.ascontiguousarray(
            np.concatenate([bq[h0], bq[h1]]).reshape(128, 1).astype(f32)
        )
        bo2 = np.ascontiguousarray(
            np.stack(
                [bo + np.tile(bv[h], H) @ Wo for h in (h0, h1)]
            ).astype(NP_AD)
        )
        in_maps.append(
            dict(
                qT=qTn, kT=kTn, vT=vTn,
                wq=pack(Wq), wk=pack(Wk), wv=pack(Wv),
                bq=bq_p, wo=wo_r, bo2=bo2, eye=eye, ones=ones,
            )
        )
    return in_maps


def assemble_output(results):
    out = np.empty((H * 256, 1024), np.float32)
    for c in range(NCORES):
        yc = results[c]["y"]  # [2, B, 128, 1024]
        for hl in range(2):
            h = 2 * c + hl
            out[h * 256 : (h + 1) * 256] = yc[hl].reshape(256, 1024)
    return out.reshape(B, S, D)


_NC_CACHE = {}
_CALLABLE_CACHE = {}


def _build_callable(nc):
    """Jit the bass module once (no donation; kernel writes all of y)."""
    import jax
    from jax.sharding import Mesh, PartitionSpec
    from jax.experimental.shard_map import shard_map
    from concourse import bass2jax
    import concourse.mybir as mb

    bass2jax.install_neuronx_cc_hook()
    pname = nc.partition_id_tensor.name if nc.partition_id_tensor else None
    in_names, out_names, out_avals = [], [], []
    for alloc in nc.m.functions[0].allocations:
        if not isinstance(alloc, mb.MemoryLocationSet):
            continue
        name = alloc.memorylocations[0].name
        if alloc.kind == "ExternalInput":
            if name != pname:
                in_names.append(name)
        elif alloc.kind == "ExternalOutput":
            out_names.append(name)
            out_avals.append(jax.core.ShapedArray(
                tuple(alloc.tensor_shape), mb.dt.np(alloc.dtype)))
    all_in = list(in_names) + list(out_names) + ([pname] if pname else [])
    zero_outs = [np.zeros(a.shape, a.dtype) for a in out_avals]

    def _body(*args):
        operands = list(args)
        if pname is not None:
            operands.append(bass2jax.partition_id_tensor())
        return tuple(bass2jax._bass_exec_p.bind(
            *operands, out_avals=tuple(out_avals), in_names=tuple(all_in),
            out_names=tuple(out_names), lowering_input_output_aliases=(),
            sim_require_finite=True, sim_require_nnan=True, nc=nc))

    mesh = Mesh(np.asarray(jax.devices()[:NCORES]), ("core",))
    # qT/kT/vT/eye/ones/wo are identical on every core: replicate (one
    # host->device transfer) instead of concatenating 8 copies
    shared = {"qT", "kT", "vT", "eye", "ones", "wo"}
    specs_in = tuple(
        PartitionSpec() if n in shared else PartitionSpec("core")
        for n in in_names
    ) + (PartitionSpec("core"),) * len(out_names)
    f = jax.jit(
        shard_map(_body, mesh=mesh, in_specs=specs_in,
                  out_specs=(PartitionSpec("core"),) * len(out_names),
                  check_rep=False),
        keep_unused=True,
    )
    return f, in_names, out_names, zero_outs, shared


def run(inputs, trace=False, reps=1):
    if reps not in _NC_CACHE:
        _NC_CACHE[reps] = build_nc(reps)
    nc = _NC_CACHE[reps]
    in_maps = prep_inputs(**inputs)
    try:
        import jax

        if reps not in _CALLABLE_CACHE:
            _CALLABLE_CACHE[reps] = _build_callable(nc)
        f, in_names, out_names, zero_outs, shared = _CALLABLE_CACHE[reps]
        concat_in = [
            np.asarray(in_maps[0][n]) if n in shared
            else np.concatenate([np.asarray(in_maps[c][n]) for c in range(NCORES)], 0)
            for n in in_names
        ] + [np.zeros((NCORES * z.shape[0], *z.shape[1:]), z.dtype)
             for z in zero_outs]
        out_arrs = [np.asarray(a) for a in f(*concat_in)]
        per_core = [
            a.reshape(NCORES, a.shape[0] // NCORES, *a.shape[1:])
            for a in out_arrs
        ]
        results = [
            {n: per_core[i][c] for i, n in enumerate(out_names)}
            for c in range(NCORES)
        ]

        class R:
            exec_time_ns = None
        return assemble_output(results), R()
    except Exception:
        from concourse.bass_utils import run_bass_kernel_spmd

        r = run_bass_kernel_spmd(nc, in_maps, list(range(NCORES)), trace=trace)
        return assemble_output(r.results), r


def kernel(**inputs) -> np.ndarray:
    out, _ = run(inputs, trace=False)
    return out
